# revision 1
# baseline (speedup 1.0000x reference)
"""Single-head attention on Trainium2: out = softmax(x Wq (x Wk)^T / sqrt(64)) (x Wv).

Full inputs: x [8, 2048, 512], Wq/Wk/Wv [512, 64]. Data-parallel over batch:
core b computes batch element b. Per core:
  - lead prologue (groups 0-1): DMA x chunk -> TensorE transposes to x^T ->
    separate M=64 q/k/v projections (k^T lands at partitions 0-63 directly,
    no SBUF->SBUF DMA on the critical path).
  - deferred prologue (groups 2-3 + natural-v half 1) runs through the "b1"
    PSUM slot (idle until the h=1 accumulator is needed), interleaved into
    the first main-loop iterations; tag "a" PSUM stays exclusive to the
    main loop's double-buffered score tiles.
  - main loop, q-half outer / k-tile inner: S^T = k q^T (float32r matmuls),
    exp on ScalarE (scale=1/8 folded in), PV accumulation
    out^T += [v|1]^T P^T emitted one iteration late (software pipeline);
    the ones row accumulates softmax denominators. Per-half out^T has its
    own 2-bank PSUM region so the half-0 tail overlaps the half-1 loop.
  - tail per half: copy out^T to SBUF, TensorE-transpose back to natural
    layout, multiply by reciprocal denominators, DMA out per quarter.
"""

import numpy as np

B, S, E, D = 8, 2048, 512, 64
NCORES = 8
NT = S // 128   # 16 s-tiles
NE = E // 128   # 4 e-chunks
NG = 4          # row groups of 512 (4 s-tiles each)
NH = 2          # q-halves of 1024
SCALE = 1.0 / float(np.sqrt(D))

_CACHE = {}


def _build():
    import concourse.bass as bass
    import concourse.tile as tile
    from concourse import bacc, mybir
    from concourse.masks import make_identity

    f32 = mybir.dt.float32
    f32r = mybir.dt.float32r
    AF = mybir.ActivationFunctionType

    nc = bacc.Bacc("TRN2", target_bir_lowering=False, debug=False,
                   num_devices=NCORES)

    x_d = nc.dram_tensor("x", [S, E], f32r, kind="ExternalInput").ap()
    wq_d = nc.dram_tensor("Wq", [E, D], f32r, kind="ExternalInput").ap()
    wk_d = nc.dram_tensor("Wk", [E, D], f32r, kind="ExternalInput").ap()
    wv_d = nc.dram_tensor("Wv", [E, D], f32r, kind="ExternalInput").ap()
    out_d = nc.dram_tensor("out", [S, D], f32, kind="ExternalOutput").ap()

    with tile.TileContext(nc) as tc:
        with (
            tc.tile_pool(name="persist", bufs=1) as pp,
            tc.tile_pool(name="ptp", bufs=3) as ptp,
            tc.tile_pool(name="small", bufs=4) as sp,
            tc.tile_pool(name="ps", bufs=1, space="PSUM") as ps,
        ):
            ident = pp.tile([128, 128], f32)
            make_identity(nc, ident[:])
            identr = pp.tile([128, 128], f32r)
            nc.vector.tensor_copy(identr[:], ident[:])

            wqk_s = pp.tile([128, NE, 2 * D], f32r)
            wv_s = pp.tile([128, NE, D], f32r)

            # x in 8 half-chunks (2 s-tiles each) alternating HWDGE rings;
            # the weight loads ride the scalar ring behind its first chunk
            x_r = x_d.rearrange("(t p) e -> p t e", p=128)
            x_c = {}
            for g in range(NG):
                for hf in range(2):
                    xc = pp.tile([128, 2, E], f32r, name=f"x_c{g}_{hf}",
                                 tag=f"x_c{g}_{hf}")
                    eng = nc.sync if hf == 0 else nc.scalar
                    eng.dma_start(xc[:],
                                  x_r[:, g * 4 + hf * 2: g * 4 + hf * 2 + 2, :])
                    x_c[(g, hf)] = xc
                if g == 0:
                    nc.scalar.dma_start(
                        wqk_s[:, :, 0:D],
                        wq_d.rearrange("(p a) d -> p a d", a=NE))
                    nc.scalar.dma_start(
                        wqk_s[:, :, D:2 * D],
                        wk_d.rearrange("(p a) d -> p a d", a=NE))
                elif g == 1:
                    nc.scalar.dma_start(
                        wv_s[:], wv_d.rearrange("(p a) d -> p a d", a=NE))

            # preload the exp ACT table off the critical path
            dummy = sp.tile([128, 1], f32, name="dummy")
            nc.scalar.activation(dummy[:], ident[:, 0:1], AF.Exp)

            xT_g, qkT_g, kT_g, vT_g = [], [], [], []
            for g in range(NG):
                xT_g.append(pp.tile([128, NE, 512], f32r, name=f"xT_g{g}",
                                    tag=f"xT_g{g}"))
                qkT_g.append(pp.tile([128, 512], f32r, name=f"qkT_g{g}",
                                     tag=f"qkT_g{g}"))
                kT_g.append(pp.tile([64, 512], f32r, name=f"kT_g{g}",
                                    tag=f"kT_g{g}"))
                vT_g.append(pp.tile([128, 512], f32r, name=f"vT_g{g}",
                                    tag=f"vT_g{g}"))
            q2_g = [pp.tile([128, 512], f32r, name=f"q2_g{g}",
                            tag=f"q2_g{g}") for g in range(NG)]
            kh_g = [pp.tile([128, 512], f32r, name=f"kh_g{g}",
                            tag=f"kh_g{g}") for g in range(2)]
            v_sb = []
            for hb in range(2):
                vs = pp.tile([128, 8, D + 1], f32r, name=f"v_sb{hb}",
                             tag=f"v_sb{hb}")
                nc.gpsimd.memset(vs[:, :, D:D + 1].bitcast(f32), 1.0)
                v_sb.append(vs)

            def emit_transposes(g, ep, tag, dve_only):
                # ep = st-pair index; consumes only half-chunk x_c[(g, ep)]
                pst = ps.tile([128, 1024], f32r, tag=tag,
                              bufs=2 if tag == "a" else 1,
                              name=f"xtp{g}_{ep}")
                for a in range(NE):
                    for stl in range(2):
                        nc.tensor.transpose(
                            pst[:, a * 256 + stl * 128: a * 256 + (stl + 1) * 128],
                            x_c[(g, ep)][:, stl, :].rearrange(
                                "p (ee a) -> p a ee", a=NE)[:, a, :],
                            identr[:],
                        )
                for ai in range(2):
                    # two copies of 2 e-phases each: [128, 512]
                    dst = xT_g[g].rearrange(
                        "p a (sp s) -> p a sp s", sp=2)[:, 2 * ai:2 * ai + 2, ep, :]
                    srcc = pst[:, ai * 512:(ai + 1) * 512].rearrange(
                        "p (a s) -> p a s", a=2)
                    if dve_only or (g + ep + ai) % 2 == 1:
                        nc.vector.tensor_copy(dst, srcc)
                    else:
                        nc.scalar.copy(dst, srcc)

            def emit_proj_sep(g, q_first):
                """Lead groups: separate M=64 projections, k^T at parts 0-63
                without a DMA. pk borrows the b1 slot (idle pre-main)."""
                order = ["q", "k"] if q_first else ["k", "q"]
                pj = ps.tile([128, 1024], f32, tag="b0", bufs=1,
                             name=f"projs{g}")
                pk = ps.tile([64, 512], f32, tag="b1", bufs=1, name=f"projk{g}")
                for what in order:
                    if what == "k":
                        for ec in range(NE):
                            nc.tensor.matmul(
                                pk[:, :], wqk_s[:, ec, D:2 * D],
                                xT_g[g][:, ec, :],
                                start=(ec == 0), stop=(ec == NE - 1),
                            )
                        nc.vector.tensor_copy(kT_g[g][:], pk[:, :])
                    else:
                        for ec in range(NE):
                            nc.tensor.matmul(
                                pj[0:64, 0:512], wqk_s[:, ec, 0:D],
                                xT_g[g][:, ec, :],
                                start=(ec == 0), stop=(ec == NE - 1),
                            )
                        nc.vector.tensor_copy(qkT_g[g][0:64, :], pj[0:64, 0:512])
                for ec in range(NE):
                    nc.tensor.matmul(
                        pj[0:64, 512:1024], wv_s[:, ec, :], xT_g[g][:, ec, :],
                        start=(ec == 0), stop=(ec == NE - 1),
                    )
                nc.scalar.copy(vT_g[g][0:64, :], pj[0:64, 512:1024])
                # hi-partition mirrors for row-tiled scores (HW concurrency)
                nc.sync.dma_start(q2_g[g][64:128, :], qkT_g[g][0:64, :])
                nc.sync.dma_start(kh_g[g][64:128, :], kT_g[g][:])

            def emit_proj_packed(g):
                """Deferred groups: packed [Wq|Wk] + Wv in the b1 slot;
                k^T moved to partitions 0-63 by SBUF->SBUF DMA (has slack)."""
                pj = ps.tile([128, 1024], f32, tag="b1", bufs=1,
                             name=f"projp{g}")
                for ec in range(NE):
                    nc.tensor.matmul(
                        pj[:, 0:512], wqk_s[:, ec, :], xT_g[g][:, ec, :],
                        start=(ec == 0), stop=(ec == NE - 1),
                    )
                for ec in range(NE):
                    nc.tensor.matmul(
                        pj[0:64, 512:1024], wv_s[:, ec, :], xT_g[g][:, ec, :],
                        start=(ec == 0), stop=(ec == NE - 1),
                    )
                nc.vector.tensor_copy(qkT_g[g][:], pj[:, 0:512])
                nc.vector.tensor_copy(vT_g[g][0:64, :], pj[0:64, 512:1024])
                nc.scalar.dma_start(kT_g[g][:], qkT_g[g][64:128, :])
                nc.sync.dma_start(q2_g[g][64:128, :], qkT_g[g][0:64, :])

            def emit_vnat(hb, part, tag):
                """part=None: all 8 tiles; part=0/1: 4-tile halves."""
                js = list(range(8) if part is None else
                          range(part * 4, (part + 1) * 4))
                width = 128 * len(js)
                vnp = ps.tile([128, width], f32r, tag=tag, bufs=1,
                              name=f"vnat{hb}_{part}")
                for i, j in enumerate(js):
                    st = hb * 8 + j
                    nc.tensor.transpose(
                        vnp[:, i * 128: i * 128 + D],
                        vT_g[st // 4][0:64, (st % 4) * 128:(st % 4 + 1) * 128],
                        identr[0:D, 0:D],
                    )
                nc.vector.tensor_copy(
                    v_sb[hb][:, js[0]:js[-1] + 1, 0:D],
                    vnp.rearrange("p (t c) -> p t c", c=128)[:, 0:len(js), 0:D],
                )

            out_r = out_d.rearrange("(t p) d -> p t d", p=128)
            outT = {}
            st8 = {"pending": None}

            def emit_scores_exp(h, kt):
                g = kt // 4
                ksl = slice((kt % 4) * 128, (kt % 4 + 1) * 128)
                khi = kh_g[g] if g < 2 else qkT_g[g]
                sT = ps.tile([128, 1024], f32, tag="a", bufs=2,
                             name=f"sT{h}_{kt}")
                nc.tensor.matmul(
                    sT[:, 0:512],
                    kT_g[g][:, ksl],
                    qkT_g[2 * h][0:64, :],
                    start=True, stop=True,
                )
                nc.tensor.matmul(
                    sT[:, 512:1024],
                    khi[64:128, ksl],
                    q2_g[2 * h + 1][64:128, :],
                    start=True, stop=True,
                )
                pT = ptp.tile([128, 1024], f32r, name="pT")
                nc.scalar.activation(pT[:], sT[:], AF.Exp, scale=SCALE)
                return pT

            def emit_pv(h, kt, pT):
                for sc in range(2):
                    nc.tensor.matmul(
                        outT[h][:, sc * 512:(sc + 1) * 512],
                        v_sb[kt // 8][:, kt % 8, :],
                        pT[:, sc * 512:(sc + 1) * 512],
                        start=(kt == 0), stop=(kt == NT - 1),
                        skip_group_check=True,
                    )

            def emit_main_iter(h, kt):
                pT = emit_scores_exp(h, kt)
                if st8["pending"] is not None:
                    emit_pv(*st8["pending"])
                st8["pending"] = (h, kt, pT)

            def emit_tail(h, dve_only=False):
                outTh_sb = pp.tile([D + 1, 1024], f32, name=f"outTsb{h}",
                                   tag=f"outTsb{h}")
                nat = ps.tile([128, 1024], f32, tag=f"b{h}", bufs=1,
                              name=f"nat{h}")
                lrec = sp.tile([128, 8], f32, name=f"lrec{h}", tag=f"lrec{h}")
                out_sbh = pp.tile([128, 8, D], f32, name=f"out_sb{h}",
                                  tag=f"out_sb{h}")
                for sc in range(2):
                    dst = outTh_sb[:, sc * 512:(sc + 1) * 512]
                    src = outT[h][:, sc * 512:(sc + 1) * 512]
                    if dve_only or sc % 2 == 1:
                        nc.vector.tensor_copy(dst, src)
                    else:
                        nc.scalar.copy(dst, src)
                    for jj in range(4):
                        j = sc * 4 + jj
                        nc.tensor.transpose(
                            nat[:, j * 128: j * 128 + D + 1],
                            outTh_sb[:, j * 128:(j + 1) * 128],
                            ident[0:D + 1, 0:D + 1],
                        )
                    nc.vector.reciprocal(
                        lrec[:, sc * 4:(sc + 1) * 4],
                        nat.rearrange("p (t c) -> p t c", c=128)[:, sc * 4:(sc + 1) * 4, D],
                    )
                    for jj in range(4):
                        j = sc * 4 + jj
                        if dve_only or jj % 2 == 1:
                            nc.vector.tensor_scalar_mul(
                                out_sbh[:, j, :],
                                nat[:, j * 128: j * 128 + D],
                                lrec[:, j:j + 1])
                        else:
                            nc.scalar.activation(out_sbh[:, j, :],
                                                 nat[:, j * 128: j * 128 + D],
                                                 AF.Copy, scale=lrec[:, j:j + 1])
                    nc.sync.dma_start(
                        out_r[:, h * 8 + sc * 4: h * 8 + (sc + 1) * 4, :],
                        out_sbh[:, sc * 4:(sc + 1) * 4, :])

            # ---- lead prologue: groups 0-1 ----
            emit_transposes(0, 0, tag="a", dve_only=False)
            emit_transposes(0, 1, tag="a", dve_only=False)
            emit_proj_sep(0, q_first=False)
            emit_transposes(1, 0, tag="a", dve_only=False)
            emit_transposes(1, 1, tag="a", dve_only=False)
            emit_proj_sep(1, q_first=True)
            emit_vnat(0, None, tag="b1")

            # ---- main h=0; deferred prologue through the b1 slot ----
            outT[0] = ps.tile([D + 1, 1024], f32, tag="b0", bufs=1,
                              name="outT0")
            filler = [
                lambda: emit_transposes(2, 0, tag="b1", dve_only=True),
                lambda: emit_transposes(2, 1, tag="b1", dve_only=True),
                lambda: emit_proj_packed(2),
                lambda: emit_vnat(1, 0, tag="b1"),
                lambda: emit_transposes(3, 0, tag="b1", dve_only=True),
                lambda: emit_transposes(3, 1, tag="b1", dve_only=True),
                lambda: emit_proj_packed(3),
                lambda: emit_vnat(1, 1, tag="b1"),
            ]
            for kt in range(NT):
                emit_main_iter(0, kt)
                if kt < len(filler):
                    filler[kt]()

            # ---- main h=1; h0's last PV flushes at kt=0, tail0 overlaps ----
            outT[1] = ps.tile([D + 1, 1024], f32, tag="b1", bufs=1,
                              name="outT1")
            for kt in range(NT):
                emit_main_iter(1, kt)
                if kt == 1:
                    emit_tail(0, dve_only=True)
            emit_pv(*st8["pending"])
            emit_tail(1)

    nc.compile()
    return nc


def kernel(**inputs):
    from concourse.bass_utils import run_bass_kernel_spmd

    x = np.ascontiguousarray(np.asarray(inputs["x"], dtype=np.float32))
    wq = np.ascontiguousarray(np.asarray(inputs["Wq"], dtype=np.float32))
    wk = np.ascontiguousarray(np.asarray(inputs["Wk"], dtype=np.float32))
    wv = np.ascontiguousarray(np.asarray(inputs["Wv"], dtype=np.float32))

    if "nc" not in _CACHE:
        _CACHE["nc"] = _build()
    nc = _CACHE["nc"]

    in_maps = [
        {"x": np.ascontiguousarray(x[b]), "Wq": wq, "Wk": wk, "Wv": wv}
        for b in range(B)
    ]
    res = run_bass_kernel_spmd(nc, in_maps, core_ids=list(range(NCORES)))
    _CACHE["last_results"] = res
    out = np.stack([res.results[b]["out"] for b in range(B)], axis=0)
    return out



# revision 49
# speedup vs baseline: 1.2896x; 1.2896x over previous
"""Single-head attention on Trainium2: out = softmax(x Wq (x Wk)^T / sqrt(64)) (x Wv).

Full inputs: x [8, 2048, 512], Wq/Wk/Wv [512, 64]. Data-parallel over batch:
core b computes batch element b. Per core (cost-model-driven schedule):

  - x streams in via HWDGE; PE warms its pstate ramp on identity transposes
    while the first DMA is in flight. Weights ride the same ring, wq/wk early.
  - Per-s-tile x^T transposes through single-bank PSUM slots (the tile
    scheduler serializes at tile granularity, so producer/consumer pairs
    never share a slot-tile), ACT/DVE copy PSUM->SBUF.
  - Loop restructured per q-GROUP: four [65,512] single-bank PSUM
    accumulators out^T[gq] (tags b0L/b0H/b1L/b1H, the latter two created
    late so the deferred prologue can use those banks first). The exp/score
    unit is a (gq, kt-pair): two [128,512] score matmuls into one 2-bank sT
    tile, one [128,1024] exp. Pair order interleaves gq 0/1 for k-tiles 0-7,
    then gq 0/1 for k-tiles 8-15, then gq 2, gq 3 - so the loop starts as
    soon as group 0's q/k projections exist (~10us), and group 1-3
    projections + deferred transposes + v-transposes all run as fillers
    inside the loop's PE slack.
  - PV accumulation runs a deep software pipeline (pend FIFO, pops
    scheduled per pair) so the exp->PV->scores chain never gates the
    1038ns/pair exp cadence; the pipeline drains 3/pair over the last four
    pairs.
  - Lead groups 0-1: separate M=64 q/k/v projections (k^T at partitions
    0-63; scores vs k-groups 0/1 entirely on low partitions). Deferred
    groups 2-3: packed [Wq|Wk] projections (q lo / k hi; scores vs k-groups
    2/3 on high partitions via SBUF->SBUF q mirrors, hidden in loop slack).
  - Tail per q-group (copy out^T, TensorE transpose back, scale by
    reciprocal denominators from the appended ones row, DMA out): groups
    0-2 ride inside the loop; group 3's tail is split into two parallel
    DVE/ACT lanes after a split final exp.
"""

import numpy as np

B, S, E, D = 8, 2048, 512, 64
NCORES = 8
NT = S // 128   # 16 s-tiles
NE = E // 128   # 4 e-chunks
NG = 4          # row groups of 512 (4 s-tiles each)
SCALE = 1.0 / float(np.sqrt(D))
N_WARM = 10     # pstate-ramp warmup transposes before real work

_CACHE = {}


def _build():
    import concourse.bass as bass
    import concourse.tile as tile
    from concourse import bacc, mybir
    from concourse.masks import make_identity

    f32 = mybir.dt.float32
    f32r = mybir.dt.float32r
    AF = mybir.ActivationFunctionType

    nc = bacc.Bacc("TRN2", target_bir_lowering=False, debug=False,
                   num_devices=NCORES)

    x_d = nc.dram_tensor("x", [S, E], f32r, kind="ExternalInput").ap()
    wq_d = nc.dram_tensor("Wq", [E, D], f32r, kind="ExternalInput").ap()
    wk_d = nc.dram_tensor("Wk", [E, D], f32r, kind="ExternalInput").ap()
    wv_d = nc.dram_tensor("Wv", [E, D], f32r, kind="ExternalInput").ap()
    out_d = nc.dram_tensor("out", [S, D], f32, kind="ExternalOutput").ap()

    with tile.TileContext(nc) as tc:
        with (
            tc.tile_pool(name="persist", bufs=1) as pp,
            tc.tile_pool(name="ptp", bufs=8) as ptp,
            tc.tile_pool(name="small", bufs=4) as sp,
            tc.tile_pool(name="ps", bufs=1, space="PSUM") as ps,
        ):
            ident = pp.tile([128, 128], f32)
            make_identity(nc, ident[:])
            identr = pp.tile([128, 128], f32r)
            nc.vector.tensor_copy(identr[:], ident[:])

            wqk_s = pp.tile([128, NE, 2 * D], f32r)
            wv_s = pp.tile([128, NE, D], f32r)

            # x: first two s-tiles individually (fast start), then 2-tile
            # pairs; wq/wk after pair 1, wv after pair 3.
            x_r = x_d.rearrange("(t p) e -> p t e", p=128)
            x_c = {}
            xc0 = pp.tile([128, 2, E], f32r, name="x_c0", tag="x_c0")
            nc.sync.dma_start(xc0[:, 0, :], x_r[:, 0:1, :])
            nc.sync.dma_start(xc0[:, 1, :], x_r[:, 1:2, :])
            x_c[0] = xc0
            xc1 = pp.tile([128, 2, E], f32r, name="x_c1", tag="x_c1")
            nc.sync.dma_start(xc1[:, 0, :], x_r[:, 2:3, :])
            nc.sync.dma_start(xc1[:, 1, :], x_r[:, 3:4, :])
            x_c[1] = xc1
            # contiguous W staging (1KB descriptors, half the DMA time of a
            # strided store into wqk_s), engine copies do the interleave
            wq_tmp = pp.tile([128, NE * D], f32r, name="wq_tmp", tag="wq_tmp")
            wk_tmp = pp.tile([128, NE * D], f32r, name="wk_tmp", tag="wk_tmp")
            nc.sync.dma_start(
                wq_tmp[:], wq_d.rearrange("(p a) d -> p (a d)", a=NE))
            nc.sync.dma_start(
                wk_tmp[:], wk_d.rearrange("(p a) d -> p (a d)", a=NE))
            for pr in range(2, 8):
                xc = pp.tile([128, 2, E], f32r, name=f"x_c{pr}", tag=f"x_c{pr}")
                x_c[pr] = xc
                if pr <= 4:
                    nc.sync.dma_start(xc[:], x_r[:, 2 * pr:2 * pr + 2, :])
                if pr == 3:
                    nc.sync.dma_start(
                        wv_s[:], wv_d.rearrange("(p a) d -> p a d", a=NE))

            # preload the exp ACT table off the critical path
            dummy = sp.tile([128, 1], f32, name="dummy")
            nc.scalar.activation(dummy[:], ident[:, 0:1], AF.Exp)
            nc.scalar.copy(wqk_s[:, :, 0:D],
                           wq_tmp.rearrange("p (a d) -> p a d", a=NE))
            nc.vector.tensor_copy(wqk_s[:, :, D:2 * D],
                                  wk_tmp.rearrange("p (a d) -> p a d", a=NE))

            xT_g = [pp.tile([128, NE, 512], f32r, name=f"xT_g{g}",
                            tag=f"xT_g{g}") for g in range(NG)]
            qkT_g = [pp.tile([128, 512], f32r, name=f"qkT_g{g}",
                             tag=f"qkT_g{g}") for g in range(NG)]
            kT_g = {0: pp.tile([64, 512], f32r, name="kT_g0", tag="kT_g0")}
            q2_g = [pp.tile([128, 512], f32r, name=f"q2_g{g}",
                            tag=f"q2_g{g}") for g in range(NG)]
            vT_g = [pp.tile([64, 512], f32r, name=f"vT_g{g}",
                            tag=f"vT_g{g}") for g in range(NG)]
            v_sb = []
            for hb in range(2):
                vs = pp.tile([128, 8, D + 1], f32r, name=f"v_sb{hb}",
                             tag=f"v_sb{hb}")
                nc.gpsimd.memset(vs[:, :, D:D + 1].bitcast(f32), 1.0)
                v_sb.append(vs)

            # PSUM: tag "a" = 2 bufs x [128,1024] (4 banks: warmup, lead x^T
            # transposes, then sT double-buffer); b0L/b0H/b1L/b1H = 1 bank
            # each (projections + deferred work, then the outT accumulators).
            warm_tile = ps.tile([128, 1024], f32, tag="a", bufs=2,
                                name="warm")

            def emit_warm(n):
                for i in range(n):
                    j = i % 8
                    nc.tensor.transpose(
                        warm_tile[:, j * 128:(j + 1) * 128], ident[:],
                        ident[:])

            def copy_via(eng, dst, src):
                if eng == "s":
                    nc.scalar.copy(dst, src)
                else:
                    nc.vector.tensor_copy(dst, src)

            # b1L/b1H slot alternation for everything deferred
            slot_i = [0]

            def next_slot():
                slot_i[0] += 1
                return ("b1L", "b1H")[slot_i[0] % 2]

            # ---- x^T: one s-tile per PSUM slot-tile ----
            def emit_tp_tile(t, tag, eng):
                pr, stl = t // 2, t % 2
                g, sub = t // 4, t % 4
                pst = ps.tile([128, 512], f32r, tag=tag,
                              bufs=2 if tag == "a" else 1, name=f"tpt{t}")
                for a in range(NE):
                    nc.tensor.transpose(
                        pst[:, a * 128:(a + 1) * 128],
                        x_c[pr][:, stl, :].rearrange(
                            "p (ee a) -> p a ee", a=NE)[:, a, :],
                        identr[:],
                    )
                dst = xT_g[g].rearrange(
                    "p a (sp s) -> p a sp s", sp=4)[:, :, sub, :]
                src = pst.rearrange("p (a s) -> p a s", a=NE)
                if eng == "sv":
                    copy_via("s", dst[:, 0:2, :], src[:, 0:2, :])
                    copy_via("v", dst[:, 2:4, :], src[:, 2:4, :])
                else:
                    copy_via(eng, dst, src)

            def emit_tp_pair2(pr, eng):
                emit_tp_tile(2 * pr, next_slot(), eng)
                emit_tp_tile(2 * pr + 1, next_slot(), eng)

            # ---- lead projection pieces (separate q/k/v; q,k at parts 0-63,
            # v at parts 64-127) ----
            def emit_proj_q(g, tag, split):
                pj = ps.tile([64, 512], f32, tag=tag, bufs=1, name=f"pjq{g}")
                for ec in range(NE):
                    nc.tensor.matmul(
                        pj[:, :], wqk_s[:, ec, 0:D], xT_g[g][:, ec, :],
                        start=(ec == 0), stop=(ec == NE - 1))
                if split:
                    copy_via("s", qkT_g[g][0:64, 0:256], pj[:, 0:256])
                    copy_via("v", qkT_g[g][0:64, 256:512], pj[:, 256:512])
                else:
                    copy_via("s", qkT_g[g][0:64, :], pj[:, :])
                nc.sync.dma_start(q2_g[g][64:128, :], qkT_g[g][0:64, :])

            def emit_proj_k(g, tag):
                pj = ps.tile([64, 512], f32, tag=tag, bufs=1, name=f"pjk{g}")
                for ec in range(NE):
                    nc.tensor.matmul(
                        pj[:, :], wqk_s[:, ec, D:2 * D], xT_g[g][:, ec, :],
                        start=(ec == 0), stop=(ec == NE - 1))
                copy_via("s", kT_g[g][:, 0:256], pj[:, 0:256])
                copy_via("v", kT_g[g][:, 256:512], pj[:, 256:512])

            def emit_proj_v(g, tag, eng="v"):
                pj = ps.tile([64, 512], f32, tag=tag, bufs=1, name=f"pjv{g}")
                for ec in range(NE):
                    nc.tensor.matmul(
                        pj[:, :], wv_s[:, ec, :], xT_g[g][:, ec, :],
                        start=(ec == 0), stop=(ec == NE - 1))
                copy_via(eng, vT_g[g][:, :], pj[:, :])

            # ---- deferred packed projections: [Wq|Wk] -> q lo / k hi ----
            proj_ps = {}

            def emit_proj_packed_qk(g, tag, part):
                if part == 0:
                    proj_ps[("qk", g)] = ps.tile([128, 512], f32, tag=tag,
                                                 bufs=1, name=f"pjqk{g}")
                pj = proj_ps[("qk", g)]
                for ec in ((0, 1) if part == 0 else (2, 3)):
                    nc.tensor.matmul(
                        pj[:, :], wqk_s[:, ec, :], xT_g[g][:, ec, :],
                        start=(ec == 0), stop=(ec == NE - 1))
                if part == 1:
                    copy_via("v", qkT_g[g][:], pj[:, :])
                    nc.sync.dma_start(q2_g[g][64:128, :], qkT_g[g][0:64, :])

            def emit_proj_packed_v(g, tag, part):
                if part == 0:
                    proj_ps[("v", g)] = ps.tile([64, 512], f32, tag=tag,
                                                bufs=1, name=f"pjv{g}")
                pj = proj_ps[("v", g)]
                for ec in ((0, 1) if part == 0 else (2, 3)):
                    nc.tensor.matmul(
                        pj[:, :], wv_s[:, ec, :], xT_g[g][:, ec, :],
                        start=(ec == 0), stop=(ec == NE - 1))
                if part == 1:
                    copy_via("v", vT_g[g][:, :], pj[:, :])

            # ---- v natural: 2 s-tiles at a time through a 1-bank slot ----
            def emit_vnat(duo, tag, eng="v"):
                js = range(duo * 2, (duo + 1) * 2)
                vnp = ps.tile([128, 256], f32r, tag=tag, bufs=1,
                              name=f"vnat{duo}")
                for i, st in enumerate(js):
                    nc.tensor.transpose(
                        vnp[:, i * 128:i * 128 + D],
                        vT_g[st // 4][:, (st % 4) * 128:(st % 4 + 1) * 128],
                        identr[0:D, 0:D],
                    )
                copy_via(eng,
                         v_sb[duo // 4][:, (duo % 4) * 2:(duo % 4) * 2 + 2,
                                        0:D],
                         vnp.rearrange("p (t c) -> p t c", c=128)[:, :, 0:D])

            # ---- main loop ----
            out_r = out_d.rearrange("(t p) d -> p t d", p=128)
            OTAG = ["b0L", "b0H", "b1L", "b1H"]
            outT = {}
            pend = []

            def get_outT(gq):
                if gq not in outT:
                    outT[gq] = ps.tile([D + 1, 512], f32, tag=OTAG[gq],
                                       bufs=1, name=f"outT{gq}")
                return outT[gq]

            def emit_pv(gq, kt, pT, half):
                nc.tensor.matmul(
                    get_outT(gq)[:, :],
                    v_sb[kt // 8][:, kt % 8, :],
                    pT[:, half * 512:(half + 1) * 512],
                    start=(kt == 0), stop=(kt == NT - 1),
                    skip_group_check=True,
                )

            def emit_pair(gq, ktp, pops, split_exp=False):
                sT = ps.tile([128, 1024], f32, tag="a", bufs=2,
                             name=f"sT{gq}_{ktp}")
                for half in range(2):
                    kt = 2 * ktp + half
                    gk = kt // 4
                    ksl = slice((kt % 4) * 128, (kt % 4 + 1) * 128)
                    if gk == 0:
                        stat = kT_g[0][:, ksl]
                        mv = qkT_g[gq][0:64, :]
                    else:
                        stat = qkT_g[gk][64:128, ksl]
                        mv = q2_g[gq][64:128, :]
                    nc.tensor.matmul(sT[:, half * 512:(half + 1) * 512],
                                     stat, mv, start=True, stop=True)
                pT = ptp.tile([128, 1024], f32r, name="pT")
                if split_exp:
                    nc.scalar.activation(pT[:, 0:512], sT[:, 0:512],
                                         AF.Exp, scale=SCALE)
                    pend.append((gq, 2 * ktp, pT, 0))
                    for _ in range(min(pops, len(pend))):
                        emit_pv(*pend.pop(0))
                    nc.scalar.activation(pT[:, 512:1024], sT[:, 512:1024],
                                         AF.Exp, scale=SCALE)
                    pend.append((gq, 2 * ktp + 1, pT, 1))
                else:
                    nc.scalar.activation(pT[:], sT[:], AF.Exp, scale=SCALE)
                    pend.append((gq, 2 * ktp, pT, 0))
                    pend.append((gq, 2 * ktp + 1, pT, 1))
                    for _ in range(pops):
                        emit_pv(*pend.pop(0))

            # ---- tail ----
            outTh_sb = [pp.tile([D + 1, 512], f32r, name=f"outTsb{i}",
                                tag=f"outTsb{i}") for i in range(2)]
            osb_lane = [pp.tile([D + 1, 256], f32r, name=f"osbl{i}",
                                tag=f"osbl{i}") for i in range(2)]
            lrec = sp.tile([128, 4, 4], f32, name="lrec", tag="lrec")
            lrec_ln = [sp.tile([128, 2], f32, name=f"lrecl{i}",
                               tag=f"lrecl{i}") for i in range(2)]
            out_sbg = {}

            def emit_tail(gq, lanes=1):
                """Flush outT[gq] (4 s-tiles): copy, transpose back, scale
                by 1/denominator, DMA out."""
                if lanes == 1:
                    osb = outTh_sb[gq % 2]
                    out_sbg[gq] = pp.tile([128, 4, D], f32,
                                          name=f"out_sb{gq}",
                                          tag=f"out_sb{gq}")
                    nat = ps.tile([128, 4 * (D + 1)], f32r, tag=OTAG[gq],
                                  bufs=1, name=f"nat{gq}")
                    natv = nat.rearrange("p (t c) -> p t c", c=D + 1)
                    copy_via("v", osb[:, :], outT[gq][:, :])
                    for jj in range(4):
                        nc.tensor.transpose(
                            nat[:, jj * (D + 1):(jj + 1) * (D + 1)],
                            osb[:, jj * 128:(jj + 1) * 128],
                            identr[0:D + 1, 0:D + 1])
                    nc.vector.reciprocal(lrec[:, gq, 0:4], natv[:, :, D])
                    for jj in range(4):
                        nc.vector.tensor_scalar_mul(
                            out_sbg[gq][:, jj, :], natv[:, jj, 0:D],
                            lrec[:, gq, jj:jj + 1])
                    nc.sync.dma_start(out_r[:, gq * 4:gq * 4 + 4, :],
                                      out_sbg[gq][:, :, :])
                    return
                # two fully independent 2-tile lanes (disjoint tiles so the
                # tile-granular scheduler never serializes across engines):
                # lane 0 on DVE, lane 1 on ACT (recips/scale-muls on DVE)
                for ln, eng in ((0, "v"), (1, "v")):
                    osb = osb_lane[ln]
                    osg = pp.tile([128, 2, D], f32, name=f"out_sb{gq}_{ln}",
                                  tag=f"out_sb{gq}_{ln}")
                    # use the long-retired b0 banks so neither lane's nat
                    # write-after-read blocks on the other lane's copy
                    nat = ps.tile([128, 2 * (D + 1)], f32r,
                                  tag=("b0L", "b0H")[ln],
                                  bufs=1, name=f"nat{gq}_{ln}")
                    natv = nat.rearrange("p (t c) -> p t c", c=D + 1)
                    csl = slice(ln * 256, (ln + 1) * 256)
                    copy_via("v", osb[:, :], outT[gq][:, csl])
                    for jj in range(2):
                        nc.tensor.transpose(
                            nat[:, jj * (D + 1):(jj + 1) * (D + 1)],
                            osb[:, jj * 128:(jj + 1) * 128],
                            identr[0:D + 1, 0:D + 1])
                    nc.vector.reciprocal(lrec_ln[ln][:, 0:2], natv[:, :, D])
                    for jj in range(2):
                        if eng == "s":
                            nc.scalar.activation(
                                osg[:, jj, :], natv[:, jj, 0:D],
                                AF.Copy, scale=lrec_ln[ln][:, jj:jj + 1])
                        else:
                            nc.vector.tensor_scalar_mul(
                                osg[:, jj, :], natv[:, jj, 0:D],
                                lrec_ln[ln][:, jj:jj + 1])
                    nc.sync.dma_start(
                        out_r[:, gq * 4 + 2 * ln:gq * 4 + 2 * ln + 2, :],
                        osg[:, :, :])

            # ================= emission =================
            emit_warm(N_WARM)
            emit_tp_tile(0, "a", "s")
            emit_tp_tile(1, "a", "v")
            emit_tp_tile(2, "a", "s")
            emit_tp_tile(3, "a", "v")
            emit_proj_q(0, "b0L", split=True)
            # x pairs 5-7 ride behind the q mirror so the mirror's transfer
            # isn't queued after the whole x stream on the DMA engines
            for pr in range(5, 8):
                nc.sync.dma_start(x_c[pr][:], x_r[:, 2 * pr:2 * pr + 2, :])
            emit_proj_k(0, "b0H")
            emit_tp_tile(4, next_slot(), "s")
            emit_tp_tile(5, next_slot(), "v")

            # fillers per pair index (deadline-scheduled: each deferred chain
            # starts at its earliest x-arrival, projections feed scores two
            # pairs ahead, vnats feed the pop schedule)
            fillers = {
                0: [lambda: emit_tp_tile(6, next_slot(), "s"),
                    lambda: emit_tp_tile(7, next_slot(), "v")],
                1: [lambda: emit_proj_packed_qk(1, "b0L", 0),
                    lambda: emit_proj_packed_qk(1, "b0L", 1)],
                2: [lambda: emit_tp_pair2(4, "v"),
                    lambda: emit_tp_pair2(5, "v")],
                3: [lambda: emit_proj_v(0, next_slot(), "s"),
                    lambda: emit_proj_packed_qk(2, next_slot(), 0)],
                4: [lambda: emit_proj_packed_qk(2, next_slot(), 1),
                    lambda: emit_proj_v(1, next_slot(), "s")],
                5: [lambda: emit_vnat(0, next_slot()),
                    lambda: emit_vnat(1, next_slot())],
                6: [lambda: emit_vnat(2, next_slot()),
                    lambda: emit_vnat(3, next_slot())],
                7: [lambda: emit_proj_packed_v(2, next_slot(), 0),
                    lambda: emit_proj_packed_v(2, next_slot(), 1)],
                8: [lambda: emit_tp_pair2(6, "v"),
                    lambda: emit_tp_pair2(7, "v")],
                9: [lambda: emit_proj_packed_qk(3, next_slot(), 0),
                    lambda: emit_proj_packed_qk(3, next_slot(), 1)],
                10: [lambda: emit_vnat(4, next_slot()),
                     lambda: emit_vnat(5, next_slot())],
                12: [lambda: emit_proj_packed_v(3, next_slot(), 0),
                     lambda: emit_proj_packed_v(3, next_slot(), 1)],
                14: [lambda: emit_vnat(6, next_slot()),
                     lambda: emit_vnat(7, next_slot())],
            }

            # pair sequence (gq, ktp) + pops per pair
            pairs = [(0, 0), (0, 1), (1, 0), (1, 1),
                     (0, 2), (0, 3), (1, 2), (1, 3),
                     (0, 4), (0, 5), (1, 4), (1, 5),
                     (0, 6), (0, 7), (1, 6), (1, 7)]
            pairs += [(2, k) for k in range(8)] + [(3, k) for k in range(8)]
            pops_sched = [0] * 6 + [4, 4] + [2] * 20 + [4] * 4
            for pi, (gq, ktp) in enumerate(pairs):
                last = (pi == len(pairs) - 1)
                emit_pair(gq, ktp, pops_sched[pi], split_exp=last)
                for f in fillers.get(pi, []):
                    f()
                if pi == 17:
                    emit_tail(0)
                elif pi == 19:
                    emit_tail(1)
                elif pi == 27:
                    emit_tail(2)
            while pend:
                emit_pv(*pend.pop(0))
            emit_tail(3, lanes=2)

    nc.compile()
    return nc


def kernel(**inputs):
    from concourse.bass_utils import run_bass_kernel_spmd

    x = np.ascontiguousarray(np.asarray(inputs["x"], dtype=np.float32))
    wq = np.ascontiguousarray(np.asarray(inputs["Wq"], dtype=np.float32))
    wk = np.ascontiguousarray(np.asarray(inputs["Wk"], dtype=np.float32))
    wv = np.ascontiguousarray(np.asarray(inputs["Wv"], dtype=np.float32))

    if "nc" not in _CACHE:
        _CACHE["nc"] = _build()
    nc = _CACHE["nc"]

    in_maps = [
        {"x": np.ascontiguousarray(x[b]), "Wq": wq, "Wk": wk, "Wv": wv}
        for b in range(B)
    ]
    res = run_bass_kernel_spmd(nc, in_maps, core_ids=list(range(NCORES)))
    _CACHE["last_results"] = res
    out = np.stack([res.results[b]["out"] for b in range(B)], axis=0)
    return out


# revision 50
# speedup vs baseline: 1.2972x; 1.0059x over previous
"""Single-head attention on Trainium2: out = softmax(x Wq (x Wk)^T / sqrt(64)) (x Wv).

Full inputs: x [8, 2048, 512], Wq/Wk/Wv [512, 64]. Data-parallel over batch:
core b computes batch element b. Per core (cost-model-driven schedule):

  - x streams in via HWDGE; PE warms its pstate ramp on identity transposes
    while the first DMA is in flight. Weights ride the same ring, wq/wk early.
  - Per-s-tile x^T transposes through single-bank PSUM slots (the tile
    scheduler serializes at tile granularity, so producer/consumer pairs
    never share a slot-tile), ACT/DVE copy PSUM->SBUF.
  - Loop restructured per q-GROUP: four [65,512] single-bank PSUM
    accumulators out^T[gq] (tags b0L/b0H/b1L/b1H, the latter two created
    late so the deferred prologue can use those banks first). The exp/score
    unit is a (gq, kt-pair): two [128,512] score matmuls into one 2-bank sT
    tile, one [128,1024] exp. Pair order interleaves gq 0/1 for k-tiles 0-7,
    then gq 0/1 for k-tiles 8-15, then gq 2, gq 3 - so the loop starts as
    soon as group 0's q/k projections exist (~10us), and group 1-3
    projections + deferred transposes + v-transposes all run as fillers
    inside the loop's PE slack.
  - PV accumulation runs a deep software pipeline (pend FIFO, pops
    scheduled per pair) so the exp->PV->scores chain never gates the
    1038ns/pair exp cadence; the pipeline drains 3/pair over the last four
    pairs.
  - Lead groups 0-1: separate M=64 q/k/v projections (k^T at partitions
    0-63; scores vs k-groups 0/1 entirely on low partitions). Deferred
    groups 2-3: packed [Wq|Wk] projections (q lo / k hi; scores vs k-groups
    2/3 on high partitions via SBUF->SBUF q mirrors, hidden in loop slack).
  - Tail per q-group (copy out^T, TensorE transpose back, scale by
    reciprocal denominators from the appended ones row, DMA out): groups
    0-2 ride inside the loop; group 3's tail is split into two parallel
    DVE/ACT lanes after a split final exp.
"""

import numpy as np

B, S, E, D = 8, 2048, 512, 64
NCORES = 8
NT = S // 128   # 16 s-tiles
NE = E // 128   # 4 e-chunks
NG = 4          # row groups of 512 (4 s-tiles each)
SCALE = 1.0 / float(np.sqrt(D))
N_WARM = 10     # pstate-ramp warmup transposes before real work

_CACHE = {}


def _build():
    import concourse.bass as bass
    import concourse.tile as tile
    from concourse import bacc, mybir
    from concourse.masks import make_identity

    f32 = mybir.dt.float32
    f32r = mybir.dt.float32r
    AF = mybir.ActivationFunctionType

    nc = bacc.Bacc("TRN2", target_bir_lowering=False, debug=False,
                   num_devices=NCORES)

    x_d = nc.dram_tensor("x", [S, E], f32r, kind="ExternalInput").ap()
    wq_d = nc.dram_tensor("Wq", [E, D], f32r, kind="ExternalInput").ap()
    wk_d = nc.dram_tensor("Wk", [E, D], f32r, kind="ExternalInput").ap()
    wv_d = nc.dram_tensor("Wv", [E, D], f32r, kind="ExternalInput").ap()
    out_d = nc.dram_tensor("out", [S, D], f32, kind="ExternalOutput").ap()

    with tile.TileContext(nc) as tc:
        with (
            tc.tile_pool(name="persist", bufs=1) as pp,
            tc.tile_pool(name="ptp", bufs=12) as ptp,
            tc.tile_pool(name="small", bufs=4) as sp,
            tc.tile_pool(name="ps", bufs=1, space="PSUM") as ps,
        ):
            ident = pp.tile([128, 128], f32)
            make_identity(nc, ident[:])
            identr = pp.tile([128, 128], f32r)
            nc.vector.tensor_copy(identr[:], ident[:])

            wqk_s = pp.tile([128, NE, 2 * D], f32r)
            wv_s = pp.tile([128, NE, D], f32r)

            # x: first two s-tiles individually (fast start), then 2-tile
            # pairs; wq/wk after pair 1, wv after pair 3.
            x_r = x_d.rearrange("(t p) e -> p t e", p=128)
            x_c = {}
            xc0 = pp.tile([128, 2, E], f32r, name="x_c0", tag="x_c0")
            nc.sync.dma_start(xc0[:, 0, :], x_r[:, 0:1, :])
            nc.sync.dma_start(xc0[:, 1, :], x_r[:, 1:2, :])
            x_c[0] = xc0
            xc1 = pp.tile([128, 2, E], f32r, name="x_c1", tag="x_c1")
            nc.sync.dma_start(xc1[:, 0, :], x_r[:, 2:3, :])
            nc.sync.dma_start(xc1[:, 1, :], x_r[:, 3:4, :])
            x_c[1] = xc1
            # contiguous W staging (1KB descriptors, half the DMA time of a
            # strided store into wqk_s), engine copies do the interleave
            wq_tmp = pp.tile([128, NE * D], f32r, name="wq_tmp", tag="wq_tmp")
            wk_tmp = pp.tile([128, NE * D], f32r, name="wk_tmp", tag="wk_tmp")
            nc.sync.dma_start(
                wq_tmp[:], wq_d.rearrange("(p a) d -> p (a d)", a=NE))
            nc.sync.dma_start(
                wk_tmp[:], wk_d.rearrange("(p a) d -> p (a d)", a=NE))
            for pr in range(2, 8):
                xc = pp.tile([128, 2, E], f32r, name=f"x_c{pr}", tag=f"x_c{pr}")
                x_c[pr] = xc
                if pr <= 4:
                    nc.sync.dma_start(xc[:], x_r[:, 2 * pr:2 * pr + 2, :])
                if pr == 3:
                    nc.sync.dma_start(
                        wv_s[:], wv_d.rearrange("(p a) d -> p a d", a=NE))

            # preload the exp ACT table off the critical path
            dummy = sp.tile([128, 1], f32, name="dummy")
            nc.scalar.activation(dummy[:], ident[:, 0:1], AF.Exp)
            nc.scalar.copy(wqk_s[:, :, 0:D],
                           wq_tmp.rearrange("p (a d) -> p a d", a=NE))
            nc.vector.tensor_copy(wqk_s[:, :, D:2 * D],
                                  wk_tmp.rearrange("p (a d) -> p a d", a=NE))

            xT_g = [pp.tile([128, NE, 512], f32r, name=f"xT_g{g}",
                            tag=f"xT_g{g}") for g in range(NG)]
            qkT_g = [pp.tile([128, 512], f32r, name=f"qkT_g{g}",
                             tag=f"qkT_g{g}") for g in range(NG)]
            kT_g = {0: pp.tile([64, 512], f32r, name="kT_g0", tag="kT_g0")}
            q2_g = [pp.tile([128, 512], f32r, name=f"q2_g{g}",
                            tag=f"q2_g{g}") for g in range(NG)]
            vT_g = [pp.tile([64, 512], f32r, name=f"vT_g{g}",
                            tag=f"vT_g{g}") for g in range(NG)]
            v_sb = []
            for hb in range(2):
                vs = pp.tile([128, 8, D + 1], f32r, name=f"v_sb{hb}",
                             tag=f"v_sb{hb}")
                nc.gpsimd.memset(vs[:, :, D:D + 1].bitcast(f32), 1.0)
                v_sb.append(vs)

            # PSUM: tag "a" = 2 bufs x [128,1024] (4 banks: warmup, lead x^T
            # transposes, then sT double-buffer); b0L/b0H/b1L/b1H = 1 bank
            # each (projections + deferred work, then the outT accumulators).
            warm_tile = ps.tile([128, 1024], f32, tag="a", bufs=2,
                                name="warm")

            def emit_warm(n):
                for i in range(n):
                    j = i % 8
                    nc.tensor.transpose(
                        warm_tile[:, j * 128:(j + 1) * 128], ident[:],
                        ident[:])

            def copy_via(eng, dst, src):
                if eng == "s":
                    nc.scalar.copy(dst, src)
                else:
                    nc.vector.tensor_copy(dst, src)

            # b1L/b1H slot alternation for everything deferred
            slot_i = [0]

            def next_slot():
                slot_i[0] += 1
                return ("b1L", "b1H")[slot_i[0] % 2]

            # ---- x^T: one s-tile per PSUM slot-tile ----
            def emit_tp_tile(t, tag, eng):
                pr, stl = t // 2, t % 2
                g, sub = t // 4, t % 4
                pst = ps.tile([128, 512], f32r, tag=tag,
                              bufs=2 if tag == "a" else 1, name=f"tpt{t}")
                for a in range(NE):
                    nc.tensor.transpose(
                        pst[:, a * 128:(a + 1) * 128],
                        x_c[pr][:, stl, :].rearrange(
                            "p (ee a) -> p a ee", a=NE)[:, a, :],
                        identr[:],
                    )
                dst = xT_g[g].rearrange(
                    "p a (sp s) -> p a sp s", sp=4)[:, :, sub, :]
                src = pst.rearrange("p (a s) -> p a s", a=NE)
                if eng == "sv":
                    copy_via("s", dst[:, 0:2, :], src[:, 0:2, :])
                    copy_via("v", dst[:, 2:4, :], src[:, 2:4, :])
                else:
                    copy_via(eng, dst, src)

            def emit_tp_pair2(pr, eng):
                emit_tp_tile(2 * pr, next_slot(), eng)
                emit_tp_tile(2 * pr + 1, next_slot(), eng)

            # ---- lead projection pieces (separate q/k/v; q,k at parts 0-63,
            # v at parts 64-127) ----
            def emit_proj_q(g, tag, split):
                pj = ps.tile([64, 512], f32, tag=tag, bufs=1, name=f"pjq{g}")
                for ec in range(NE):
                    nc.tensor.matmul(
                        pj[:, :], wqk_s[:, ec, 0:D], xT_g[g][:, ec, :],
                        start=(ec == 0), stop=(ec == NE - 1))
                if split:
                    copy_via("s", qkT_g[g][0:64, 0:256], pj[:, 0:256])
                    copy_via("v", qkT_g[g][0:64, 256:512], pj[:, 256:512])
                else:
                    copy_via("s", qkT_g[g][0:64, :], pj[:, :])
                nc.sync.dma_start(q2_g[g][64:128, :], qkT_g[g][0:64, :])

            def emit_proj_k(g, tag):
                pj = ps.tile([64, 512], f32, tag=tag, bufs=1, name=f"pjk{g}")
                for ec in range(NE):
                    nc.tensor.matmul(
                        pj[:, :], wqk_s[:, ec, D:2 * D], xT_g[g][:, ec, :],
                        start=(ec == 0), stop=(ec == NE - 1))
                copy_via("s", kT_g[g][:, 0:256], pj[:, 0:256])
                copy_via("v", kT_g[g][:, 256:512], pj[:, 256:512])

            def emit_proj_v(g, tag, eng="v"):
                pj = ps.tile([64, 512], f32, tag=tag, bufs=1, name=f"pjv{g}")
                for ec in range(NE):
                    nc.tensor.matmul(
                        pj[:, :], wv_s[:, ec, :], xT_g[g][:, ec, :],
                        start=(ec == 0), stop=(ec == NE - 1))
                copy_via(eng, vT_g[g][:, :], pj[:, :])

            # ---- deferred packed projections: [Wq|Wk] -> q lo / k hi ----
            proj_ps = {}

            def emit_proj_packed_qk(g, tag, part):
                if part == 0:
                    proj_ps[("qk", g)] = ps.tile([128, 512], f32, tag=tag,
                                                 bufs=1, name=f"pjqk{g}")
                pj = proj_ps[("qk", g)]
                for ec in ((0, 1) if part == 0 else (2, 3)):
                    nc.tensor.matmul(
                        pj[:, :], wqk_s[:, ec, :], xT_g[g][:, ec, :],
                        start=(ec == 0), stop=(ec == NE - 1))
                if part == 1:
                    copy_via("v", qkT_g[g][:], pj[:, :])
                    nc.sync.dma_start(q2_g[g][64:128, :], qkT_g[g][0:64, :])

            def emit_proj_packed_v(g, tag, part):
                if part == 0:
                    proj_ps[("v", g)] = ps.tile([64, 512], f32, tag=tag,
                                                bufs=1, name=f"pjv{g}")
                pj = proj_ps[("v", g)]
                for ec in ((0, 1) if part == 0 else (2, 3)):
                    nc.tensor.matmul(
                        pj[:, :], wv_s[:, ec, :], xT_g[g][:, ec, :],
                        start=(ec == 0), stop=(ec == NE - 1))
                if part == 1:
                    copy_via("v", vT_g[g][:, :], pj[:, :])

            # ---- v natural: 2 s-tiles at a time through a 1-bank slot ----
            def emit_vnat(duo, tag, eng="v"):
                js = range(duo * 2, (duo + 1) * 2)
                vnp = ps.tile([128, 256], f32r, tag=tag, bufs=1,
                              name=f"vnat{duo}")
                for i, st in enumerate(js):
                    nc.tensor.transpose(
                        vnp[:, i * 128:i * 128 + D],
                        vT_g[st // 4][:, (st % 4) * 128:(st % 4 + 1) * 128],
                        identr[0:D, 0:D],
                    )
                copy_via(eng,
                         v_sb[duo // 4][:, (duo % 4) * 2:(duo % 4) * 2 + 2,
                                        0:D],
                         vnp.rearrange("p (t c) -> p t c", c=128)[:, :, 0:D])

            # ---- main loop ----
            out_r = out_d.rearrange("(t p) d -> p t d", p=128)
            OTAG = ["b0L", "b0H", "b1L", "b1H"]
            outT = {}
            pend = []

            def get_outT(gq):
                if gq not in outT:
                    outT[gq] = ps.tile([D + 1, 512], f32, tag=OTAG[gq],
                                       bufs=1, name=f"outT{gq}")
                return outT[gq]

            def emit_pv(gq, kt, pT, half):
                nc.tensor.matmul(
                    get_outT(gq)[:, :],
                    v_sb[kt // 8][:, kt % 8, :],
                    pT[:, half * 512:(half + 1) * 512],
                    start=(kt == 0), stop=(kt == NT - 1),
                    skip_group_check=True,
                )

            def emit_pair(gq, ktp, pops, split_exp=False):
                sT = ps.tile([128, 1024], f32, tag="a", bufs=2,
                             name=f"sT{gq}_{ktp}")
                for half in range(2):
                    kt = 2 * ktp + half
                    gk = kt // 4
                    ksl = slice((kt % 4) * 128, (kt % 4 + 1) * 128)
                    if gk == 0:
                        stat = kT_g[0][:, ksl]
                        mv = qkT_g[gq][0:64, :]
                    else:
                        stat = qkT_g[gk][64:128, ksl]
                        mv = q2_g[gq][64:128, :]
                    nc.tensor.matmul(sT[:, half * 512:(half + 1) * 512],
                                     stat, mv, start=True, stop=True)
                pT = ptp.tile([128, 1024], f32r, name="pT")
                if split_exp:
                    nc.scalar.activation(pT[:, 0:512], sT[:, 0:512],
                                         AF.Exp, scale=SCALE)
                    pend.append((gq, 2 * ktp, pT, 0))
                    for _ in range(min(pops, len(pend))):
                        emit_pv(*pend.pop(0))
                    nc.scalar.activation(pT[:, 512:1024], sT[:, 512:1024],
                                         AF.Exp, scale=SCALE)
                    pend.append((gq, 2 * ktp + 1, pT, 1))
                else:
                    nc.scalar.activation(pT[:], sT[:], AF.Exp, scale=SCALE)
                    pend.append((gq, 2 * ktp, pT, 0))
                    pend.append((gq, 2 * ktp + 1, pT, 1))
                    for _ in range(pops):
                        emit_pv(*pend.pop(0))

            # ---- tail ----
            outTh_sb = [pp.tile([D + 1, 512], f32r, name=f"outTsb{i}",
                                tag=f"outTsb{i}") for i in range(2)]
            osb_lane = [pp.tile([D + 1, 256], f32r, name=f"osbl{i}",
                                tag=f"osbl{i}") for i in range(2)]
            lrec = sp.tile([128, 4, 4], f32, name="lrec", tag="lrec")
            lrec_ln = [sp.tile([128, 2], f32, name=f"lrecl{i}",
                               tag=f"lrecl{i}") for i in range(2)]
            out_sbg = {}

            def emit_tail(gq, lanes=1):
                """Flush outT[gq] (4 s-tiles): copy, transpose back, scale
                by 1/denominator, DMA out."""
                if lanes == 1:
                    osb = outTh_sb[gq % 2]
                    out_sbg[gq] = pp.tile([128, 4, D], f32,
                                          name=f"out_sb{gq}",
                                          tag=f"out_sb{gq}")
                    nat = ps.tile([128, 4 * (D + 1)], f32r, tag=OTAG[gq],
                                  bufs=1, name=f"nat{gq}")
                    natv = nat.rearrange("p (t c) -> p t c", c=D + 1)
                    copy_via("v", osb[:, :], outT[gq][:, :])
                    for jj in range(4):
                        nc.tensor.transpose(
                            nat[:, jj * (D + 1):(jj + 1) * (D + 1)],
                            osb[:, jj * 128:(jj + 1) * 128],
                            identr[0:D + 1, 0:D + 1])
                    nc.vector.reciprocal(lrec[:, gq, 0:4], natv[:, :, D])
                    for jj in range(4):
                        nc.vector.tensor_scalar_mul(
                            out_sbg[gq][:, jj, :], natv[:, jj, 0:D],
                            lrec[:, gq, jj:jj + 1])
                    nc.sync.dma_start(out_r[:, gq * 4:gq * 4 + 4, :],
                                      out_sbg[gq][:, :, :])
                    return
                # two fully independent 2-tile lanes (disjoint tiles so the
                # tile-granular scheduler never serializes across engines):
                # lane 0 on DVE, lane 1 on ACT (recips/scale-muls on DVE)
                for ln, eng in ((0, "v"), (1, "v")):
                    osb = osb_lane[ln]
                    osg = pp.tile([128, 2, D], f32, name=f"out_sb{gq}_{ln}",
                                  tag=f"out_sb{gq}_{ln}")
                    # use the long-retired b0 banks so neither lane's nat
                    # write-after-read blocks on the other lane's copy
                    nat = ps.tile([128, 2 * (D + 1)], f32r,
                                  tag=("b0L", "b0H")[ln],
                                  bufs=1, name=f"nat{gq}_{ln}")
                    natv = nat.rearrange("p (t c) -> p t c", c=D + 1)
                    csl = slice(ln * 256, (ln + 1) * 256)
                    copy_via("v", osb[:, :], outT[gq][:, csl])
                    for jj in range(2):
                        nc.tensor.transpose(
                            nat[:, jj * (D + 1):(jj + 1) * (D + 1)],
                            osb[:, jj * 128:(jj + 1) * 128],
                            identr[0:D + 1, 0:D + 1])
                    nc.vector.reciprocal(lrec_ln[ln][:, 0:2], natv[:, :, D])
                    for jj in range(2):
                        if eng == "s":
                            nc.scalar.activation(
                                osg[:, jj, :], natv[:, jj, 0:D],
                                AF.Copy, scale=lrec_ln[ln][:, jj:jj + 1])
                        else:
                            nc.vector.tensor_scalar_mul(
                                osg[:, jj, :], natv[:, jj, 0:D],
                                lrec_ln[ln][:, jj:jj + 1])
                    nc.sync.dma_start(
                        out_r[:, gq * 4 + 2 * ln:gq * 4 + 2 * ln + 2, :],
                        osg[:, :, :])

            # ================= emission =================
            emit_warm(N_WARM)
            emit_tp_tile(0, "a", "s")
            emit_tp_tile(1, "a", "v")
            emit_tp_tile(2, "a", "s")
            emit_tp_tile(3, "a", "v")
            emit_proj_q(0, "b0L", split=True)
            # x pairs 5-7 ride behind the q mirror so the mirror's transfer
            # isn't queued after the whole x stream on the DMA engines
            for pr in range(5, 8):
                nc.sync.dma_start(x_c[pr][:], x_r[:, 2 * pr:2 * pr + 2, :])
            emit_proj_k(0, "b0H")
            emit_tp_tile(4, next_slot(), "s")
            emit_tp_tile(5, next_slot(), "v")

            # fillers per pair index (deadline-scheduled: each deferred chain
            # starts at its earliest x-arrival, projections feed scores two
            # pairs ahead, vnats feed the pop schedule)
            fillers = {
                0: [lambda: emit_tp_tile(6, next_slot(), "s"),
                    lambda: emit_tp_tile(7, next_slot(), "v")],
                1: [lambda: emit_proj_packed_qk(1, "b0L", 0),
                    lambda: emit_proj_packed_qk(1, "b0L", 1)],
                2: [lambda: emit_tp_pair2(4, "v"),
                    lambda: emit_tp_pair2(5, "v")],
                3: [lambda: emit_proj_v(0, next_slot(), "s"),
                    lambda: emit_proj_packed_qk(2, next_slot(), 0)],
                4: [lambda: emit_proj_packed_qk(2, next_slot(), 1),
                    lambda: emit_proj_v(1, next_slot(), "s")],
                5: [lambda: emit_vnat(0, next_slot()),
                    lambda: emit_vnat(1, next_slot())],
                6: [lambda: emit_vnat(2, next_slot()),
                    lambda: emit_vnat(3, next_slot())],
                7: [lambda: emit_proj_packed_v(2, next_slot(), 0),
                    lambda: emit_proj_packed_v(2, next_slot(), 1)],
                8: [lambda: emit_tp_pair2(6, "v"),
                    lambda: emit_tp_pair2(7, "v")],
                9: [lambda: emit_proj_packed_qk(3, next_slot(), 0),
                    lambda: emit_proj_packed_qk(3, next_slot(), 1)],
                10: [lambda: emit_vnat(4, next_slot()),
                     lambda: emit_vnat(5, next_slot())],
                12: [lambda: emit_proj_packed_v(3, next_slot(), 0),
                     lambda: emit_proj_packed_v(3, next_slot(), 1)],
                14: [lambda: emit_vnat(6, next_slot()),
                     lambda: emit_vnat(7, next_slot())],
            }

            # pair sequence (gq, ktp) + pops per pair
            pairs = [(0, 0), (0, 1), (1, 0), (1, 1),
                     (0, 2), (0, 3), (1, 2), (1, 3),
                     (0, 4), (0, 5), (1, 4), (1, 5),
                     (0, 6), (0, 7), (1, 6), (1, 7)]
            pairs += [(2, k) for k in range(8)] + [(3, k) for k in range(8)]
            pops_sched = ([0] * 6 + [2] * 8 + [3] * 9 + [2] * 5
                          + [3, 3, 2, 3])
            for pi, (gq, ktp) in enumerate(pairs):
                last = (pi == len(pairs) - 1)
                emit_pair(gq, ktp, pops_sched[pi], split_exp=last)
                for f in fillers.get(pi, []):
                    f()
                if pi == 17:
                    emit_tail(0)
                elif pi == 19:
                    emit_tail(1)
                elif pi == 25:
                    emit_tail(2)
            while pend:
                emit_pv(*pend.pop(0))
            emit_tail(3, lanes=2)

    nc.compile()
    return nc


def kernel(**inputs):
    from concourse.bass_utils import run_bass_kernel_spmd

    x = np.ascontiguousarray(np.asarray(inputs["x"], dtype=np.float32))
    wq = np.ascontiguousarray(np.asarray(inputs["Wq"], dtype=np.float32))
    wk = np.ascontiguousarray(np.asarray(inputs["Wk"], dtype=np.float32))
    wv = np.ascontiguousarray(np.asarray(inputs["Wv"], dtype=np.float32))

    if "nc" not in _CACHE:
        _CACHE["nc"] = _build()
    nc = _CACHE["nc"]

    in_maps = [
        {"x": np.ascontiguousarray(x[b]), "Wq": wq, "Wk": wk, "Wv": wv}
        for b in range(B)
    ]
    res = run_bass_kernel_spmd(nc, in_maps, core_ids=list(range(NCORES)))
    _CACHE["last_results"] = res
    out = np.stack([res.results[b]["out"] for b in range(B)], axis=0)
    return out


# revision 51
# speedup vs baseline: 1.3237x; 1.0204x over previous
"""Single-head attention on Trainium2: out = softmax(x Wq (x Wk)^T / sqrt(64)) (x Wv).

Full inputs: x [8, 2048, 512], Wq/Wk/Wv [512, 64]. Data-parallel over batch:
core b computes batch element b. Per core (cost-model-driven schedule):

  - x streams in via HWDGE; PE warms its pstate ramp on identity transposes
    while the first DMA is in flight. Weights ride the same ring, wq/wk early.
  - Per-s-tile x^T transposes through single-bank PSUM slots (the tile
    scheduler serializes at tile granularity, so producer/consumer pairs
    never share a slot-tile), ACT/DVE copy PSUM->SBUF.
  - Loop restructured per q-GROUP: four [65,512] single-bank PSUM
    accumulators out^T[gq] (tags b0L/b0H/b1L/b1H, the latter two created
    late so the deferred prologue can use those banks first). The exp/score
    unit is a (gq, kt-pair): two [128,512] score matmuls into one 2-bank sT
    tile, one [128,1024] exp. Pair order interleaves gq 0/1 for k-tiles 0-7,
    then gq 0/1 for k-tiles 8-15, then gq 2, gq 3 - so the loop starts as
    soon as group 0's q/k projections exist (~10us), and group 1-3
    projections + deferred transposes + v-transposes all run as fillers
    inside the loop's PE slack.
  - PV accumulation runs a deep software pipeline (pend FIFO, pops
    scheduled per pair) so the exp->PV->scores chain never gates the
    1038ns/pair exp cadence; the pipeline drains 3/pair over the last four
    pairs.
  - Lead groups 0-1: separate M=64 q/k/v projections (k^T at partitions
    0-63; scores vs k-groups 0/1 entirely on low partitions). Deferred
    groups 2-3: packed [Wq|Wk] projections (q lo / k hi; scores vs k-groups
    2/3 on high partitions via SBUF->SBUF q mirrors, hidden in loop slack).
  - Tail per q-group (copy out^T, TensorE transpose back, scale by
    reciprocal denominators from the appended ones row, DMA out): groups
    0-2 ride inside the loop; group 3's tail is split into two parallel
    DVE/ACT lanes after a split final exp.
"""

import numpy as np

B, S, E, D = 8, 2048, 512, 64
NCORES = 8
NT = S // 128   # 16 s-tiles
NE = E // 128   # 4 e-chunks
NG = 4          # row groups of 512 (4 s-tiles each)
SCALE = 1.0 / float(np.sqrt(D))
N_WARM = 10     # pstate-ramp warmup transposes before real work

_CACHE = {}


def _build():
    import concourse.bass as bass
    import concourse.tile as tile
    from concourse import bacc, mybir
    from concourse.masks import make_identity

    f32 = mybir.dt.float32
    f32r = mybir.dt.float32r
    AF = mybir.ActivationFunctionType

    nc = bacc.Bacc("TRN2", target_bir_lowering=False, debug=False,
                   num_devices=NCORES)

    x_d = nc.dram_tensor("x", [S, E], f32r, kind="ExternalInput").ap()
    wq_d = nc.dram_tensor("Wq", [E, D], f32r, kind="ExternalInput").ap()
    wk_d = nc.dram_tensor("Wk", [E, D], f32r, kind="ExternalInput").ap()
    wv_d = nc.dram_tensor("Wv", [E, D], f32r, kind="ExternalInput").ap()
    out_d = nc.dram_tensor("out", [S, D], f32, kind="ExternalOutput").ap()

    with tile.TileContext(nc) as tc:
        with (
            tc.tile_pool(name="persist", bufs=1) as pp,
            tc.tile_pool(name="ptp", bufs=12) as ptp,
            tc.tile_pool(name="small", bufs=4) as sp,
            tc.tile_pool(name="ps", bufs=1, space="PSUM") as ps,
        ):
            ident = pp.tile([128, 128], f32)
            make_identity(nc, ident[:])
            identr = pp.tile([128, 128], f32r)
            nc.vector.tensor_copy(identr[:], ident[:])

            wqk_s = pp.tile([128, NE, 2 * D], f32r)
            wv_s = pp.tile([128, NE, D], f32r)

            # x: first two s-tiles individually (fast start), then 2-tile
            # pairs; wq/wk after pair 1, wv after pair 3.
            x_r = x_d.rearrange("(t p) e -> p t e", p=128)
            x_c = {}
            xc0 = pp.tile([128, 2, E], f32r, name="x_c0", tag="x_c0")
            nc.sync.dma_start(xc0[:, 0, :], x_r[:, 0:1, :])
            nc.sync.dma_start(xc0[:, 1, :], x_r[:, 1:2, :])
            x_c[0] = xc0
            xc1 = pp.tile([128, 2, E], f32r, name="x_c1", tag="x_c1")
            nc.sync.dma_start(xc1[:, 0, :], x_r[:, 2:3, :])
            nc.sync.dma_start(xc1[:, 1, :], x_r[:, 3:4, :])
            x_c[1] = xc1
            # contiguous W staging (1KB descriptors, half the DMA time of a
            # strided store into wqk_s), engine copies do the interleave
            wq_tmp = pp.tile([128, NE * D], f32r, name="wq_tmp", tag="wq_tmp")
            wk_tmp = pp.tile([128, NE * D], f32r, name="wk_tmp", tag="wk_tmp")
            nc.sync.dma_start(
                wq_tmp[:], wq_d.rearrange("(p a) d -> p (a d)", a=NE))
            nc.sync.dma_start(
                wk_tmp[:], wk_d.rearrange("(p a) d -> p (a d)", a=NE))
            for pr in range(2, 8):
                xc = pp.tile([128, 2, E], f32r, name=f"x_c{pr}", tag=f"x_c{pr}")
                x_c[pr] = xc
                if pr <= 4:
                    nc.sync.dma_start(xc[:], x_r[:, 2 * pr:2 * pr + 2, :])
                if pr == 3:
                    nc.sync.dma_start(
                        wv_s[:], wv_d.rearrange("(p a) d -> p a d", a=NE))

            # preload the exp ACT table off the critical path
            dummy = sp.tile([128, 1], f32, name="dummy")
            nc.scalar.activation(dummy[:], ident[:, 0:1], AF.Exp)
            nc.scalar.copy(wqk_s[:, :, 0:D],
                           wq_tmp.rearrange("p (a d) -> p a d", a=NE))
            nc.vector.tensor_copy(wqk_s[:, :, D:2 * D],
                                  wk_tmp.rearrange("p (a d) -> p a d", a=NE))

            xT_g = [pp.tile([128, NE, 512], f32r, name=f"xT_g{g}",
                            tag=f"xT_g{g}") for g in range(NG)]
            qkT_g = [pp.tile([128, 512], f32r, name=f"qkT_g{g}",
                             tag=f"qkT_g{g}") for g in range(NG)]
            kT_g = {0: pp.tile([64, 512], f32r, name="kT_g0", tag="kT_g0")}
            q2_g = [pp.tile([128, 512], f32r, name=f"q2_g{g}",
                            tag=f"q2_g{g}") for g in range(NG)]
            vT_g = [pp.tile([64, 512], f32r, name=f"vT_g{g}",
                            tag=f"vT_g{g}") for g in range(NG)]
            v_sb = []
            for hb in range(2):
                vs = pp.tile([128, 8, D + 1], f32r, name=f"v_sb{hb}",
                             tag=f"v_sb{hb}")
                nc.gpsimd.memset(vs[:, :, D:D + 1].bitcast(f32), 1.0)
                v_sb.append(vs)

            # PSUM: tag "a" = 2 bufs x [128,1024] (4 banks: warmup, lead x^T
            # transposes, then sT double-buffer); b0L/b0H/b1L/b1H = 1 bank
            # each (projections + deferred work, then the outT accumulators).
            warm_tile = ps.tile([128, 1024], f32, tag="a", bufs=2,
                                name="warm")

            def emit_warm(n):
                for i in range(n):
                    j = i % 8
                    nc.tensor.transpose(
                        warm_tile[:, j * 128:(j + 1) * 128], ident[:],
                        ident[:])

            def copy_via(eng, dst, src):
                if eng == "s":
                    nc.scalar.copy(dst, src)
                else:
                    nc.vector.tensor_copy(dst, src)

            # b1L/b1H slot alternation for everything deferred
            slot_i = [0]

            def next_slot():
                slot_i[0] += 1
                return ("b1L", "b1H")[slot_i[0] % 2]

            # ---- x^T: one s-tile per PSUM slot-tile ----
            def emit_tp_tile(t, tag, eng):
                pr, stl = t // 2, t % 2
                g, sub = t // 4, t % 4
                pst = ps.tile([128, 512], f32r, tag=tag,
                              bufs=2 if tag == "a" else 1, name=f"tpt{t}")
                for a in range(NE):
                    nc.tensor.transpose(
                        pst[:, a * 128:(a + 1) * 128],
                        x_c[pr][:, stl, :].rearrange(
                            "p (ee a) -> p a ee", a=NE)[:, a, :],
                        identr[:],
                    )
                dst = xT_g[g].rearrange(
                    "p a (sp s) -> p a sp s", sp=4)[:, :, sub, :]
                src = pst.rearrange("p (a s) -> p a s", a=NE)
                if eng == "sv":
                    copy_via("s", dst[:, 0:2, :], src[:, 0:2, :])
                    copy_via("v", dst[:, 2:4, :], src[:, 2:4, :])
                else:
                    copy_via(eng, dst, src)

            def emit_tp_pair2(pr, eng):
                emit_tp_tile(2 * pr, next_slot(), eng)
                emit_tp_tile(2 * pr + 1, next_slot(), eng)

            # ---- lead projection pieces (separate q/k/v; q,k at parts 0-63,
            # v at parts 64-127) ----
            def emit_proj_q(g, tag, split):
                pj = ps.tile([64, 512], f32, tag=tag, bufs=1, name=f"pjq{g}")
                for ec in range(NE):
                    nc.tensor.matmul(
                        pj[:, :], wqk_s[:, ec, 0:D], xT_g[g][:, ec, :],
                        start=(ec == 0), stop=(ec == NE - 1))
                if split:
                    copy_via("s", qkT_g[g][0:64, 0:256], pj[:, 0:256])
                    copy_via("v", qkT_g[g][0:64, 256:512], pj[:, 256:512])
                else:
                    copy_via("s", qkT_g[g][0:64, :], pj[:, :])
                nc.sync.dma_start(q2_g[g][64:128, :], qkT_g[g][0:64, :])

            def emit_proj_k(g, tag):
                pj = ps.tile([64, 512], f32, tag=tag, bufs=1, name=f"pjk{g}")
                for ec in range(NE):
                    nc.tensor.matmul(
                        pj[:, :], wqk_s[:, ec, D:2 * D], xT_g[g][:, ec, :],
                        start=(ec == 0), stop=(ec == NE - 1))
                copy_via("s", kT_g[g][:, 0:256], pj[:, 0:256])
                copy_via("v", kT_g[g][:, 256:512], pj[:, 256:512])

            def emit_proj_v(g, tag, eng="v"):
                pj = ps.tile([64, 512], f32, tag=tag, bufs=1, name=f"pjv{g}")
                for ec in range(NE):
                    nc.tensor.matmul(
                        pj[:, :], wv_s[:, ec, :], xT_g[g][:, ec, :],
                        start=(ec == 0), stop=(ec == NE - 1))
                copy_via(eng, vT_g[g][:, :], pj[:, :])

            # ---- deferred packed projections: [Wq|Wk] -> q lo / k hi ----
            proj_ps = {}

            def emit_proj_packed_qk(g, tag, part):
                if part == 0:
                    proj_ps[("qk", g)] = ps.tile([128, 512], f32, tag=tag,
                                                 bufs=1, name=f"pjqk{g}")
                pj = proj_ps[("qk", g)]
                for ec in ((0, 1) if part == 0 else (2, 3)):
                    nc.tensor.matmul(
                        pj[:, :], wqk_s[:, ec, :], xT_g[g][:, ec, :],
                        start=(ec == 0), stop=(ec == NE - 1))
                if part == 1:
                    copy_via("v", qkT_g[g][:], pj[:, :])
                    nc.sync.dma_start(q2_g[g][64:128, :], qkT_g[g][0:64, :])

            def emit_proj_packed_v(g, tag, part):
                if part == 0:
                    proj_ps[("v", g)] = ps.tile([64, 512], f32, tag=tag,
                                                bufs=1, name=f"pjv{g}")
                pj = proj_ps[("v", g)]
                for ec in ((0, 1) if part == 0 else (2, 3)):
                    nc.tensor.matmul(
                        pj[:, :], wv_s[:, ec, :], xT_g[g][:, ec, :],
                        start=(ec == 0), stop=(ec == NE - 1))
                if part == 1:
                    copy_via("v", vT_g[g][:, :], pj[:, :])

            # ---- v natural: 2 s-tiles at a time through a 1-bank slot ----
            def emit_vnat(duo, tag, eng="v"):
                js = range(duo * 2, (duo + 1) * 2)
                vnp = ps.tile([128, 256], f32r, tag=tag, bufs=1,
                              name=f"vnat{duo}")
                for i, st in enumerate(js):
                    nc.tensor.transpose(
                        vnp[:, i * 128:i * 128 + D],
                        vT_g[st // 4][:, (st % 4) * 128:(st % 4 + 1) * 128],
                        identr[0:D, 0:D],
                    )
                copy_via(eng,
                         v_sb[duo // 4][:, (duo % 4) * 2:(duo % 4) * 2 + 2,
                                        0:D],
                         vnp.rearrange("p (t c) -> p t c", c=128)[:, :, 0:D])

            # ---- main loop ----
            out_r = out_d.rearrange("(t p) d -> p t d", p=128)
            OTAG = ["b0L", "b0H", "b1L", "b1H"]
            outT = {}
            pend = []

            def get_outT(gq):
                if gq not in outT:
                    outT[gq] = ps.tile([D + 1, 512], f32, tag=OTAG[gq],
                                       bufs=1, name=f"outT{gq}")
                return outT[gq]

            def emit_pv(gq, kt, pT, half):
                nc.tensor.matmul(
                    get_outT(gq)[:, :],
                    v_sb[kt // 8][:, kt % 8, :],
                    pT[:, half * 512:(half + 1) * 512],
                    start=(kt == 0), stop=(kt == NT - 1),
                    skip_group_check=True,
                )

            def emit_pair(gq, ktp, pops, split_exp=False):
                sT = ps.tile([128, 1024], f32, tag="a", bufs=2,
                             name=f"sT{gq}_{ktp}")
                for half in range(2):
                    kt = 2 * ktp + half
                    gk = kt // 4
                    ksl = slice((kt % 4) * 128, (kt % 4 + 1) * 128)
                    if gk == 0:
                        stat = kT_g[0][:, ksl]
                        mv = qkT_g[gq][0:64, :]
                    else:
                        stat = qkT_g[gk][64:128, ksl]
                        mv = q2_g[gq][64:128, :]
                    nc.tensor.matmul(sT[:, half * 512:(half + 1) * 512],
                                     stat, mv, start=True, stop=True)
                pT = ptp.tile([128, 1024], f32r, name="pT")
                if split_exp:
                    nc.scalar.activation(pT[:, 0:512], sT[:, 0:512],
                                         AF.Exp, scale=SCALE)
                    pend.append((gq, 2 * ktp, pT, 0))
                    for _ in range(min(pops, len(pend))):
                        emit_pv(*pend.pop(0))
                    nc.scalar.activation(pT[:, 512:1024], sT[:, 512:1024],
                                         AF.Exp, scale=SCALE)
                    pend.append((gq, 2 * ktp + 1, pT, 1))
                else:
                    nc.scalar.activation(pT[:], sT[:], AF.Exp, scale=SCALE)
                    pend.append((gq, 2 * ktp, pT, 0))
                    pend.append((gq, 2 * ktp + 1, pT, 1))
                    for _ in range(pops):
                        emit_pv(*pend.pop(0))

            # ---- tail ----
            outTh_sb = [pp.tile([D + 1, 512], f32r, name=f"outTsb{i}",
                                tag=f"outTsb{i}") for i in range(2)]
            osb_lane = [pp.tile([D + 1, 256], f32r, name=f"osbl{i}",
                                tag=f"osbl{i}") for i in range(2)]
            lrec = sp.tile([128, 4, 4], f32, name="lrec", tag="lrec")
            lrec_ln = [sp.tile([128, 2], f32, name=f"lrecl{i}",
                               tag=f"lrecl{i}") for i in range(2)]
            out_sbg = {}

            def emit_tail(gq, lanes=1):
                """Flush outT[gq] (4 s-tiles): copy, transpose back, scale
                by 1/denominator, DMA out."""
                if lanes == 1:
                    osb = outTh_sb[gq % 2]
                    out_sbg[gq] = pp.tile([128, 4, D], f32,
                                          name=f"out_sb{gq}",
                                          tag=f"out_sb{gq}")
                    nat = ps.tile([128, 4 * (D + 1)], f32r, tag=OTAG[gq],
                                  bufs=1, name=f"nat{gq}")
                    natv = nat.rearrange("p (t c) -> p t c", c=D + 1)
                    copy_via("v", osb[:, :], outT[gq][:, :])
                    for jj in range(4):
                        nc.tensor.transpose(
                            nat[:, jj * (D + 1):(jj + 1) * (D + 1)],
                            osb[:, jj * 128:(jj + 1) * 128],
                            identr[0:D + 1, 0:D + 1])
                    nc.vector.reciprocal(lrec[:, gq, 0:4], natv[:, :, D])
                    for jj in range(4):
                        nc.vector.tensor_scalar_mul(
                            out_sbg[gq][:, jj, :], natv[:, jj, 0:D],
                            lrec[:, gq, jj:jj + 1])
                    nc.sync.dma_start(out_r[:, gq * 4:gq * 4 + 4, :],
                                      out_sbg[gq][:, :, :])
                    return
                # two fully independent 2-tile lanes (disjoint tiles so the
                # tile-granular scheduler never serializes across engines):
                # lane 0 on DVE, lane 1 on ACT (recips/scale-muls on DVE)
                for ln, eng in ((0, "v"), (1, "v")):
                    osb = osb_lane[ln]
                    osg = pp.tile([128, 2, D], f32, name=f"out_sb{gq}_{ln}",
                                  tag=f"out_sb{gq}_{ln}")
                    # use the long-retired b0 banks so neither lane's nat
                    # write-after-read blocks on the other lane's copy
                    nat = ps.tile([128, 2 * (D + 1)], f32r,
                                  tag=("b0L", "b0H")[ln],
                                  bufs=1, name=f"nat{gq}_{ln}")
                    natv = nat.rearrange("p (t c) -> p t c", c=D + 1)
                    csl = slice(ln * 256, (ln + 1) * 256)
                    copy_via("v", osb[:, :], outT[gq][:, csl])
                    for jj in range(2):
                        nc.tensor.transpose(
                            nat[:, jj * (D + 1):(jj + 1) * (D + 1)],
                            osb[:, jj * 128:(jj + 1) * 128],
                            identr[0:D + 1, 0:D + 1])
                    nc.vector.reciprocal(lrec_ln[ln][:, 0:2], natv[:, :, D])
                    for jj in range(2):
                        if eng == "s":
                            nc.scalar.activation(
                                osg[:, jj, :], natv[:, jj, 0:D],
                                AF.Copy, scale=lrec_ln[ln][:, jj:jj + 1])
                        else:
                            nc.vector.tensor_scalar_mul(
                                osg[:, jj, :], natv[:, jj, 0:D],
                                lrec_ln[ln][:, jj:jj + 1])
                    nc.sync.dma_start(
                        out_r[:, gq * 4 + 2 * ln:gq * 4 + 2 * ln + 2, :],
                        osg[:, :, :])

            # ================= emission =================
            emit_warm(N_WARM)
            emit_tp_tile(0, "a", "s")
            emit_tp_tile(1, "a", "v")
            emit_tp_tile(2, "a", "s")
            emit_tp_tile(3, "a", "v")
            emit_proj_q(0, "b0L", split=True)
            # x pairs 5-7 ride behind the q mirror so the mirror's transfer
            # isn't queued after the whole x stream on the DMA engines
            for pr in range(5, 8):
                nc.sync.dma_start(x_c[pr][:], x_r[:, 2 * pr:2 * pr + 2, :])
            emit_proj_k(0, "b0H")
            emit_tp_tile(4, next_slot(), "s")
            emit_tp_tile(5, next_slot(), "v")

            # fillers per pair index (deadline-scheduled: each deferred chain
            # starts at its earliest x-arrival, projections feed scores two
            # pairs ahead, vnats feed the pop schedule)
            fillers = {
                0: [lambda: emit_tp_tile(6, next_slot(), "s"),
                    lambda: emit_tp_tile(7, next_slot(), "v")],
                1: [lambda: emit_proj_packed_qk(1, "b0L", 0),
                    lambda: emit_proj_packed_qk(1, "b0L", 1)],
                2: [lambda: emit_tp_pair2(4, "v"),
                    lambda: emit_tp_pair2(5, "v")],
                3: [lambda: emit_proj_v(0, next_slot(), "s"),
                    lambda: emit_proj_packed_qk(2, next_slot(), 0)],
                4: [lambda: emit_proj_packed_qk(2, next_slot(), 1),
                    lambda: emit_proj_v(1, next_slot(), "s")],
                5: [lambda: emit_vnat(0, next_slot()),
                    lambda: emit_vnat(1, next_slot())],
                6: [lambda: emit_vnat(2, next_slot()),
                    lambda: emit_vnat(3, next_slot())],
                7: [lambda: emit_proj_packed_v(2, next_slot(), 0),
                    lambda: emit_proj_packed_v(2, next_slot(), 1)],
                8: [lambda: emit_tp_pair2(6, "v"),
                    lambda: emit_tp_pair2(7, "v")],
                9: [lambda: emit_proj_packed_qk(3, next_slot(), 0),
                    lambda: emit_proj_packed_qk(3, next_slot(), 1)],
                10: [lambda: emit_vnat(4, next_slot()),
                     lambda: emit_vnat(5, next_slot())],
                12: [lambda: emit_proj_packed_v(3, next_slot(), 0),
                     lambda: emit_proj_packed_v(3, next_slot(), 1)],
                14: [lambda: emit_vnat(6, next_slot()),
                     lambda: emit_vnat(7, next_slot())],
            }

            # pair sequence (gq, ktp) + pops per pair
            pairs = [(0, 0), (0, 1), (1, 0), (1, 1),
                     (0, 2), (0, 3), (1, 2), (1, 3),
                     (0, 4), (0, 5), (1, 4), (1, 5),
                     (0, 6), (0, 7), (1, 6), (1, 7)]
            pairs += [(2, k) for k in range(8)] + [(3, k) for k in range(8)]
            pops_sched = ([0] * 6 + [1] * 8 + [3] * 14
                          + [4, 3, 3, 4])
            for pi, (gq, ktp) in enumerate(pairs):
                last = (pi == len(pairs) - 1)
                emit_pair(gq, ktp, pops_sched[pi], split_exp=last)
                for f in fillers.get(pi, []):
                    f()
                if pi == 20:
                    emit_tail(0)
                elif pi == 21:
                    emit_tail(1)
                elif pi == 27:
                    emit_tail(2)
            while pend:
                emit_pv(*pend.pop(0))
            emit_tail(3, lanes=2)

    nc.compile()
    return nc


def kernel(**inputs):
    from concourse.bass_utils import run_bass_kernel_spmd

    x = np.ascontiguousarray(np.asarray(inputs["x"], dtype=np.float32))
    wq = np.ascontiguousarray(np.asarray(inputs["Wq"], dtype=np.float32))
    wk = np.ascontiguousarray(np.asarray(inputs["Wk"], dtype=np.float32))
    wv = np.ascontiguousarray(np.asarray(inputs["Wv"], dtype=np.float32))

    if "nc" not in _CACHE:
        _CACHE["nc"] = _build()
    nc = _CACHE["nc"]

    in_maps = [
        {"x": np.ascontiguousarray(x[b]), "Wq": wq, "Wk": wk, "Wv": wv}
        for b in range(B)
    ]
    res = run_bass_kernel_spmd(nc, in_maps, core_ids=list(range(NCORES)))
    _CACHE["last_results"] = res
    out = np.stack([res.results[b]["out"] for b in range(B)], axis=0)
    return out
